# revision 1
# baseline (speedup 1.0000x reference)
"""Trainium2 Bass kernel for the attention-scoring MLP (nn_Attn):

    enc = encoder_outputs.transpose(1,0,2)          # [B,S,Hin]
    a1  = tanh(enc @ W1_enc.T + hidden @ W1_hid.T + b1)
    s   = a1 @ W2[0] (+ b2 -- dropped: softmax shift-invariant)
    s   = where(mask, -inf, s)
    out = softmax(s, axis=-1)[:, None, :]           # [B,1,S]

Sharding: data-parallel over batch B=32 across 8 NeuronCores (4 rows
each), weights replicated, no collectives. Per core the main matmul is
computed transposed -- a1T[h, s] = W1_encT.T @ encT per batch row -- so
the (b1 + hidden@W1_hid.T) term rides the ScalarEngine's per-partition
bias port of the tanh activation, and the W2 contraction is a
PSUM-accumulated M=1 matmul over h-tiles. Matmuls run in bf16 (inputs
pre-transposed and converted host-side so all DMAs are contiguous
row-major loads); accumulation is fp32 in PSUM.
"""

import numpy as np
import ml_dtypes

import concourse.bass as bass
import concourse.tile as tile
from concourse import bacc, mybir
from concourse.bass import ds, ts
from concourse.bass_utils import run_bass_kernel_spmd
from concourse.masks import make_identity

N_CORES = 8
B, S, HIN, H = 32, 1024, 1024, 1024
BL = B // N_CORES          # local batch rows per core
P = 128                    # partitions
IT = HIN // P              # contraction tiles
HT = H // P                # output-feature tiles
NT = 512                   # moving-dim tile (s columns per matmul)
SH = S // NT               # s tiles per batch row
F32 = mybir.dt.float32
BF16 = mybir.dt.bfloat16
AF = mybir.ActivationFunctionType
BF = ml_dtypes.bfloat16

_cached_nc = None
LAST_RESULT = None  # BassKernelResults of the most recent run (for test harness)


def _build():
    global _cached_nc
    if _cached_nc is not None:
        return _cached_nc

    nc = bacc.Bacc("TRN2", target_bir_lowering=False, debug=False,
                   num_devices=N_CORES)

    # encT per batch row: [b, i, s]
    enc_ext = nc.dram_tensor("enc", [BL, HIN, S], BF16, kind="ExternalInput").ap()
    # hiddenT: [i, b]
    hidt_ext = nc.dram_tensor("hiddent", [H, BL], BF16, kind="ExternalInput").ap()
    mneg_ext = nc.dram_tensor("maskneg", [BL * S], F32, kind="ExternalInput").ap()
    # W1 split + transposed: [i, h]
    w1e_ext = nc.dram_tensor("w1e", [HIN, H], BF16, kind="ExternalInput").ap()
    w1h_ext = nc.dram_tensor("w1h", [H, H], BF16, kind="ExternalInput").ap()
    b1_ext = nc.dram_tensor("b1", [H], F32, kind="ExternalInput").ap()
    w2_ext = nc.dram_tensor("w2", [H], BF16, kind="ExternalInput").ap()
    out_ext = nc.dram_tensor("out", [BL, S], F32, kind="ExternalOutput").ap()

    with tile.TileContext(nc) as tc:
        with (
            tc.tile_pool(name="consts", bufs=1) as consts,
            tc.tile_pool(name="encp", bufs=3) as encp,
            tc.tile_pool(name="thp", bufs=7) as thp,
            tc.tile_pool(name="pap", bufs=2, space="PSUM") as pap,
            tc.tile_pool(name="pscp", bufs=2, space="PSUM") as pscp,
            tc.tile_pool(name="psA", bufs=1, space="PSUM") as psA,
            tc.tile_pool(name="psT", bufs=2, space="PSUM") as psTp,
        ):
            # ---- PE warmup: ~4us of junk matmuls with no DMA deps so the
            # HAM clock-gate is already at 8/8 when the real matmuls arrive.
            warm_sb = consts.tile([P, NT], BF16)
            nc.gpsimd.memset(warm_sb[:], 0.0)
            warm_ps = pap.tile([P, NT], F32, tag="pa1")
            for _ in range(10):
                nc.tensor.matmul(warm_ps[:], warm_sb[:, 0:P], warm_sb[:],
                                 start=True, stop=True)

            # ---- resident weights/constants ----
            # DMA emission order = ring service order: first-needed first.
            # w1e_sb[p, it*H + h] = W1[h, it*128+p]  == w1e_ext[it*128+p, h]
            # One DMA per h-tile: the ht=0 matmul group only waits for 256KB
            # of weights instead of the whole 2MB.
            w1e_t = []
            for it in range(IT):
                w = consts.tile([P, H], BF16, tag=f"w1e{it}")
                nc.sync.dma_start(w[:], w1e_ext[ds(it * P, P), :])
                w1e_t.append(w)
            hT_sb = consts.tile([P, IT * BL], BF16)
            for it in range(IT):
                nc.sync.dma_start(hT_sb[:, ts(it, BL)], hidt_ext[ds(it * P, P), :])
            # first enc block is prefetched here, before w1h (phase A can
            # wait). Split into per-it tiles so the very first matmul only
            # needs w1e[0]+enc0[0] (~256KB), not the whole 3MB preload.
            enc0_t = []
            for it in range(IT):
                e = encp.tile([P, NT], BF16, tag=f"enc0_{it}")
                nc.scalar.dma_start(e[:], enc_ext[0, ds(it * P, P), ds(0, NT)])
                enc0_t.append(e)
            w1h_t = []
            for it in range(IT):
                w = consts.tile([P, H], BF16, tag=f"w1h{it}")
                nc.scalar.dma_start(w[:], w1h_ext[ds(it * P, P), :])
                w1h_t.append(w)
            b1T_sb = consts.tile([P, HT], F32)
            nc.sync.dma_start(b1T_sb[:], b1_ext.rearrange("(ht p) -> p ht", p=P))
            w2T_sb = consts.tile([P, HT], BF16)
            nc.sync.dma_start(w2T_sb[:], w2_ext.rearrange("(ht p) -> p ht", p=P))
            mneg_sb = consts.tile([1, BL * S], F32)
            nc.sync.dma_start(mneg_sb[:], mneg_ext[:])
            ident_sb = consts.tile([BL, BL], F32)
            make_identity(nc, ident_sb[:])
            # W2 as a padded [128,128] stationary per h-tile (column 0 = w2
            # chunk, rest zero) so the scores matmul keeps the same PE config
            # as the main matmuls; only row 0 of its PSUM output is used.
            w2pad = consts.tile([P, HT * P], BF16)
            nc.gpsimd.memset(w2pad[:], 0.0)
            for ht in range(HT):
                nc.vector.tensor_copy(w2pad[:, ds(ht * P, 1)], w2T_sb[:, ds(ht, 1)])

            bias_sb = consts.tile([P, HT * BL], F32)   # [p, ht*BL+b]
            hterm_sb = consts.tile([BL, H], F32)
            scores_sb = consts.tile([1, BL * S], F32)
            c40 = consts.tile([1, 1], F32)
            nc.gpsimd.memset(c40[:], -40.0)
            exps = consts.tile([1, BL * S], F32)
            ssum = consts.tile([1, BL * SH], F32)
            rcp = consts.tile([1, BL], F32)
            attn = consts.tile([1, BL * S], F32)

            # ---- phase A: h_term[b,h] = hidden @ W1_hid.T; bias = h_termT + b1T
            pht = psA.tile([BL, H], F32)
            for it in range(IT):
                lhs = hT_sb[:, ts(it, BL)]
                nc.tensor.matmul(pht[:, 0:NT], lhs,
                                 w1h_t[it][:, ds(0, NT)],
                                 start=(it == 0), stop=(it == IT - 1))
                nc.tensor.matmul(pht[:, NT:H], lhs,
                                 w1h_t[it][:, ds(NT, NT)],
                                 start=(it == 0), stop=(it == IT - 1))
            nc.scalar.copy(hterm_sb[:], pht[:])
            for ht in range(HT):
                ptT = psTp.tile([P, BL], F32)
                nc.tensor.transpose(ptT[:], hterm_sb[:, ts(ht, P)], ident_sb[:])
                nc.vector.tensor_scalar_add(bias_sb[:, ts(ht, BL)], ptT[:],
                                            b1T_sb[:, ds(ht, 1)])

            # ---- phase B: per (b, s-half) tile of 512 sequence positions
            for t in range(BL * SH):
                b, sh = divmod(t, SH)
                # encT block: enc_sb[p, it*NT + s] = enc_ext[b, it*128+p, sh*NT+s]
                if t == 0:
                    enc_sb = None
                else:
                    enc_sb = encp.tile([P, IT * NT], BF16, tag="enc")
                    # t==1 rides the scalar ring (startup overlap with w1e on
                    # sync); steady-state tiles use the otherwise-idle sync
                    # ring so DMA triggers never serialize against tanh on ACT.
                    eng = nc.scalar if t == 1 else nc.sync
                    for it in range(IT):
                        eng.dma_start(
                            enc_sb[:, ts(it, NT)],
                            enc_ext[b, ds(it * P, P), ds(sh * NT, NT)],
                        )
                psc = pscp.tile([P, NT], F32)
                # Delay the scores matmuls so a late bias (phase A is still
                # streaming during t=0) never stalls the in-order PE.
                delay = 4 if t == 0 else (1 if t == BL * SH - 1 else 3)
                pending = []
                for ht in range(HT):
                    pa1 = pap.tile([P, NT], F32, tag="pa1")
                    for it in range(IT):
                        rhs = enc0_t[it][:] if t == 0 else enc_sb[:, ts(it, NT)]
                        nc.tensor.matmul(
                            pa1[:],
                            w1e_t[it][:, ds(ht * P, P)],
                            rhs,
                            start=(it == 0), stop=(it == IT - 1),
                        )
                    th = thp.tile([P, NT], BF16)
                    nc.scalar.activation(th[:], pa1[:], AF.Tanh,
                                         bias=bias_sb[:, ds(ht * BL + b, 1)],
                                         scale=1.0)
                    pending.append((th, ht))
                    if len(pending) > delay:
                        pth, pht_idx = pending.pop(0)
                        nc.tensor.matmul(psc[:], w2pad[:, ds(pht_idx * P, P)],
                                         pth[:],
                                         start=(pht_idx == 0),
                                         stop=(pht_idx == HT - 1))
                for pth, pht_idx in pending:
                    nc.tensor.matmul(psc[:], w2pad[:, ds(pht_idx * P, P)],
                                     pth[:], start=(pht_idx == 0),
                                     stop=(pht_idx == HT - 1))
                # scores += mask * -1e30   (scores_sb[0, t*NT:] == scores[b, sh*NT:])
                nc.vector.tensor_add(scores_sb[0:1, ds(t * NT, NT)], psc[0:1, :],
                                     mneg_sb[0:1, ds(t * NT, NT)])

                # ---- softmax, pipelined per s-half tile.
                # |scores| <= ||W2||_1 <= 32, so exp(s - 40) never overflows
                # and softmax is shift-invariant -- no max-reduce needed.
                nc.scalar.activation(exps[0:1, ds(t * NT, NT)],
                                     scores_sb[0:1, ds(t * NT, NT)],
                                     AF.Exp, bias=c40[0:1, 0:1], scale=1.0,
                                     accum_out=ssum[0:1, ds(t, 1)])
                if sh == SH - 1:
                    # total = sum of the SH per-tile partial sums for row b
                    nc.vector.reduce_sum(rcp[0:1, ds(b, 1)],
                                         ssum[0:1, ds(b * SH, SH)],
                                         axis=mybir.AxisListType.X)
                    nc.vector.reciprocal(rcp[0:1, ds(b, 1)], rcp[0:1, ds(b, 1)])
                    nc.vector.tensor_scalar_mul(attn[0:1, ds(b * S, S)],
                                                exps[0:1, ds(b * S, S)],
                                                rcp[0:1, ds(b, 1)])
                    nc.sync.dma_start(out_ext[b, :], attn[0:1, ds(b * S, S)])

    nc.compile()
    _cached_nc = nc
    return nc


def kernel(hidden, encoder_outputs, mask, W1, b1, W2, b2):
    global LAST_RESULT
    nc = _build()

    enc = np.asarray(encoder_outputs, dtype=np.float32)
    # [S,B,Hin] -> [B,Hin,S] in bf16 so per-core DMAs are contiguous
    enc_t = np.ascontiguousarray(np.transpose(enc, (1, 2, 0)).astype(BF))
    hid_t = np.ascontiguousarray(np.asarray(hidden, dtype=np.float32).T.astype(BF))  # [H,B]
    maskneg = np.where(np.asarray(mask, dtype=bool), np.float32(-1e30),
                       np.float32(0.0)).astype(np.float32)
    W1 = np.asarray(W1, dtype=np.float32)
    w1e = np.ascontiguousarray(W1[:, :HIN].T.astype(BF))   # [Hin, H]
    w1h = np.ascontiguousarray(W1[:, HIN:].T.astype(BF))   # [H, H]
    b1 = np.ascontiguousarray(np.asarray(b1, dtype=np.float32).reshape(H))
    w2 = np.ascontiguousarray(np.asarray(W2, dtype=np.float32).reshape(H).astype(BF))

    in_maps = []
    for c in range(N_CORES):
        sl = slice(c * BL, (c + 1) * BL)
        in_maps.append({
            "enc": np.ascontiguousarray(enc_t[sl]),
            "hiddent": np.ascontiguousarray(hid_t[:, sl]),
            "maskneg": np.ascontiguousarray(maskneg[sl].reshape(-1)),
            "w1e": w1e,
            "w1h": w1h,
            "b1": b1,
            "w2": w2,
        })

    res = run_bass_kernel_spmd(nc, in_maps, core_ids=list(range(N_CORES)))
    LAST_RESULT = res
    out = np.concatenate([res.results[c]["out"] for c in range(N_CORES)], axis=0)
    return np.ascontiguousarray(out[:, None, :].astype(np.float32))



# revision 4
# speedup vs baseline: 2.2933x; 2.2933x over previous
"""Trainium2 Bass kernel for the attention-scoring MLP (nn_Attn):

    enc = encoder_outputs.transpose(1,0,2)          # [B,S,Hin]
    a1  = tanh(enc @ W1_enc.T + hidden @ W1_hid.T + b1)
    s   = a1 @ W2[0] (+ b2 -- dropped: softmax shift-invariant)
    s   = where(mask, -inf, s)
    out = softmax(s, axis=-1)[:, None, :]           # [B,1,S]

Strategy (v2):
  * Data-parallel over batch B=32 across 8 NeuronCores (4 rows each),
    weights replicated, no collectives.
  * Mask packing: masked positions get attn == 0 exactly, so only the
    ~50% unmasked columns of enc are shipped/computed. Host packs each
    row's unmasked columns; device computes scores+softmax on the packed
    stream; host scatters back to [B,1,S] with zeros.  Per-row packed
    length LT = MAIN(<=512) + rem blocks; the per-row remainders beyond
    512 are batched across the 4 rows into one shared "rem" stream so
    all matmuls keep free-dim >= 128.
  * fp8 (e4m3) DoubleRow matmuls for the big enc @ W1_enc.T contraction:
    two 128-deep k-slices per instruction (~1.8x bf16). Inputs are
    scaled host-side (enc x32, W1 x2^13) to clear fp8 subnormals; the
    2^-18 compensation rides the tanh activation's scale port.  The
    hidden @ W1_hid.T + b1 term (0.1% of FLOPs) is folded host-side
    into the per-(h,b) tanh bias.
  * W2 contraction: PSUM-accumulated bf16 matmul over h-tiles on the
    tanh output (padded [128,128] stationary, row 0 of PSUM used).
  * Softmax per row on-device: exp(s-40) with accumulate, reciprocal,
    scale, DMA out the packed attn row.
"""

import numpy as np
import ml_dtypes

import concourse.bass as bass
import concourse.tile as tile
from concourse import bacc, mybir
from concourse.bass import ds, ts
from concourse.bass_utils import run_bass_kernel_spmd

N_CORES = 8
B, S, HIN, H = 32, 1024, 1024, 1024
BL = B // N_CORES          # local batch rows per core
P = 128                    # partitions
IT = HIN // P              # 128-deep contraction tiles
IT2 = IT // 2              # DoubleRow pair tiles
HT = H // P                # output-feature tiles
F32 = mybir.dt.float32
BF16 = mybir.dt.bfloat16
F8 = mybir.dt.float8e4
AF = mybir.ActivationFunctionType
DR = mybir.MatmulPerfMode.DoubleRow
BF = ml_dtypes.bfloat16
F8NP = ml_dtypes.float8_e4m3

USE_FP8 = True
ENC_SCALE = 32.0
W_SCALE = float(2.0 ** 13)
ACT_SCALE = float(1.0 / (ENC_SCALE * W_SCALE)) if USE_FP8 else 1.0
DELAY = 3                  # pending tanh->scores matmul pipeline depth

_cached = {}               # (MAIN, tuple(rem_widths)) -> compiled Bacc
LAST_RESULT = None         # BassKernelResults of the most recent run


def _geometry(lmax: int):
    """Packed-row geometry: MAIN width plus rem-block widths (x16 each)."""
    rnd = lambda x: ((x + 15) // 16) * 16
    main = min(512, rnd(lmax))
    rem = []
    off = main
    while off < lmax:
        rem.append(min(128, rnd(lmax - off)))
        off += rem[-1]
    return main, tuple(rem)


def _build(main: int, rem: tuple):
    key = (main, rem)
    if key in _cached:
        return _cached[key]

    LT = main + sum(rem)
    EDT, WDT = (F8, F8) if USE_FP8 else (BF16, BF16)

    nc = bacc.Bacc("TRN2", target_bir_lowering=False, debug=False,
                   num_devices=N_CORES)

    # encT main block per row: [b, hin, main]
    encm_ext = nc.dram_tensor("encm", [BL, HIN, main], EDT, kind="ExternalInput").ap()
    # shared rem streams: [hin, BL*rem_k]
    encr_ext = [
        nc.dram_tensor(f"encr{k}", [HIN, BL * rk], EDT, kind="ExternalInput").ap()
        for k, rk in enumerate(rem)
    ]
    # W1_enc.T packed for DoubleRow: [p, ht, it2, two, m] flattened
    w1_ext = nc.dram_tensor("w1p", [P, HT * IT * P], WDT, kind="ExternalInput").ap()
    # per-(h,b) tanh bias = (hidden @ W1_hid.T + b1).T: [p, ht*BL]
    bias_ext = nc.dram_tensor("biasT", [P, HT * BL], F32, kind="ExternalInput").ap()
    # W2 padded stationary: [p, ht*128], column 0 of each ht block = w2 chunk
    w2_ext = nc.dram_tensor("w2pad", [P, HT * P], BF16, kind="ExternalInput").ap()
    # 0 for real columns, -1e30 for padding: [BL*LT]
    pneg_ext = nc.dram_tensor("padneg", [BL * LT], F32, kind="ExternalInput").ap()
    out_ext = nc.dram_tensor("out", [BL, LT], F32, kind="ExternalOutput").ap()

    def pair(ap2d, _w):
        """View a 2D [128, 2*w] slice as the 3D [128, 2, w] DoubleRow AP."""
        return ap2d.rearrange("p (two n) -> p two n", two=2)

    with tile.TileContext(nc) as tc:
        with (
            tc.tile_pool(name="consts", bufs=1) as consts,
            tc.tile_pool(name="thp", bufs=6) as thp,
            tc.tile_pool(name="pap", bufs=2, space="PSUM") as pap,
            tc.tile_pool(name="parp", bufs=2, space="PSUM") as parp,
            tc.tile_pool(name="pscp", bufs=2, space="PSUM") as pscp,
            tc.tile_pool(name="psrp", bufs=1, space="PSUM") as psrp,
        ):
            # ---- PE warmup: junk matmuls with no DMA deps so the HAM
            # clock-gate is ramping while the first DMAs land.
            warm_sb = consts.tile([P, 512], BF16)
            nc.gpsimd.memset(warm_sb[:], 0.0)
            for _ in range(8):
                warm_ps = pap.tile([P, 512], F32, tag="pa")
                nc.tensor.matmul(warm_ps[:], warm_sb[:, 0:P], warm_sb[:],
                                 start=True, stop=True)

            # ---- resident weights/constants ----
            # DMA emission order = ring service order: first-needed first.
            w1_sb = consts.tile([P, HT * IT * P], WDT)
            for ht in range(HT):
                nc.sync.dma_start(w1_sb[:, ds(ht * IT * P, IT * P)],
                                  w1_ext[:, ds(ht * IT * P, IT * P)])
            bias_sb = consts.tile([P, HT * BL], F32)
            nc.sync.dma_start(bias_sb[:], bias_ext[:, :])
            # rem streams ride the scalar ring (parallel with w1 on sync)
            encr_sb = []
            for k, rk in enumerate(rem):
                w = BL * rk
                e = consts.tile([P, IT * w], EDT, tag=f"encr{k}")
                for it in range(IT):
                    nc.scalar.dma_start(e[:, ds(it * w, w)],
                                        encr_ext[k][ds(it * P, P), :])
                encr_sb.append(e)
            w2_sb = consts.tile([P, HT * P], BF16)
            nc.sync.dma_start(w2_sb[:], w2_ext[:, :])
            pneg_sb = consts.tile([1, BL * LT], F32)
            nc.sync.dma_start(pneg_sb[:], pneg_ext[:])
            # enc main blocks: rows 0-1 on scalar ring, rows 2-3 on sync
            encm_sb = []
            for r in range(BL):
                e = consts.tile([P, IT * main], EDT, tag=f"encm{r}")
                eng = nc.scalar if r < 2 else nc.sync
                for it in range(IT):
                    eng.dma_start(e[:, ds(it * main, main)],
                                  encm_ext[r, ds(it * P, P), :])
                encm_sb.append(e)

            scores_sb = consts.tile([1, BL * LT], F32)
            c40 = consts.tile([1, 1], F32)
            nc.gpsimd.memset(c40[:], -40.0)
            exps = consts.tile([1, BL * LT], F32)
            ssum = consts.tile([1, BL], F32)
            rcp = consts.tile([1, BL], F32)
            attn = consts.tile([1, BL * LT], F32)

            # ---- rem pass: shared remainder stream(s), all 4 rows batched.
            # Runs first so every row's full score segment is ready by the
            # time that row's main pass finishes (softmax tails pipeline).
            off = main
            for k, rk in enumerate(rem):
                w = BL * rk
                psr = psrp.tile([P, w], F32, tag=f"psr{k}")
                pend = []
                for ht in range(HT):
                    par = parp.tile([P, w], F32, tag="par")
                    for it2 in range(IT2):
                        if USE_FP8:
                            nc.tensor.matmul(
                                par[:],
                                pair(w1_sb[:, ds((ht * IT2 + it2) * 2 * P, 2 * P)], P),
                                pair(encr_sb[k][:, ds(2 * it2 * w, 2 * w)], w),
                                start=(it2 == 0), stop=(it2 == IT2 - 1),
                                perf_mode=DR)
                        else:
                            for i in range(2):
                                nc.tensor.matmul(
                                    par[:],
                                    w1_sb[:, ds(((ht * IT2 + it2) * 2 + i) * P, P)],
                                    encr_sb[k][:, ds((2 * it2 + i) * w, w)],
                                    start=(it2 == 0 and i == 0),
                                    stop=(it2 == IT2 - 1 and i == 1))
                    th = thp.tile([P, w], BF16, tag="threm")
                    for r in range(BL):
                        nc.scalar.activation(th[:, ds(r * rk, rk)],
                                             par[:, ds(r * rk, rk)], AF.Tanh,
                                             bias=bias_sb[:, ds(ht * BL + r, 1)],
                                             scale=ACT_SCALE)
                    pend.append((th, ht))
                    if len(pend) > 2:
                        pth, pht = pend.pop(0)
                        nc.tensor.matmul(psr[:], w2_sb[:, ds(pht * P, P)],
                                         pth[:], start=(pht == 0),
                                         stop=(pht == HT - 1))
                for pth, pht in pend:
                    nc.tensor.matmul(psr[:], w2_sb[:, ds(pht * P, P)], pth[:],
                                     start=(pht == 0), stop=(pht == HT - 1))
                for r in range(BL):
                    pos = r * LT + off
                    nc.vector.tensor_add(scores_sb[0:1, ds(pos, rk)],
                                         psr[0:1, ds(r * rk, rk)],
                                         pneg_sb[0:1, ds(pos, rk)])
                off += rk

            # ---- main pass: per batch row, 512-wide packed stream.
            for r in range(BL):
                psc = pscp.tile([P, main], F32, tag="psc")
                pend = []
                for ht in range(HT):
                    pa = pap.tile([P, main], F32, tag="pa")
                    for it2 in range(IT2):
                        if USE_FP8:
                            nc.tensor.matmul(
                                pa[:],
                                pair(w1_sb[:, ds((ht * IT2 + it2) * 2 * P, 2 * P)], P),
                                pair(encm_sb[r][:, ds(2 * it2 * main, 2 * main)], main),
                                start=(it2 == 0), stop=(it2 == IT2 - 1),
                                perf_mode=DR)
                        else:
                            for i in range(2):
                                nc.tensor.matmul(
                                    pa[:],
                                    w1_sb[:, ds(((ht * IT2 + it2) * 2 + i) * P, P)],
                                    encm_sb[r][:, ds((2 * it2 + i) * main, main)],
                                    start=(it2 == 0 and i == 0),
                                    stop=(it2 == IT2 - 1 and i == 1))
                    th = thp.tile([P, main], BF16, tag="th")
                    nc.scalar.activation(th[:], pa[:], AF.Tanh,
                                         bias=bias_sb[:, ds(ht * BL + r, 1)],
                                         scale=ACT_SCALE)
                    pend.append((th, ht))
                    if len(pend) > DELAY:
                        pth, pht = pend.pop(0)
                        nc.tensor.matmul(psc[:], w2_sb[:, ds(pht * P, P)],
                                         pth[:], start=(pht == 0),
                                         stop=(pht == HT - 1))
                for pth, pht in pend:
                    nc.tensor.matmul(psc[:], w2_sb[:, ds(pht * P, P)], pth[:],
                                     start=(pht == 0), stop=(pht == HT - 1))

                # ---- softmax tail for row r (rem scores already present).
                nc.vector.tensor_add(scores_sb[0:1, ds(r * LT, main)],
                                     psc[0:1, :],
                                     pneg_sb[0:1, ds(r * LT, main)])
                # |scores| <= ||w2||_1 <= 16, so exp(s - 40) never overflows
                # and softmax is shift-invariant -- no max-reduce needed.
                nc.scalar.activation(exps[0:1, ds(r * LT, LT)],
                                     scores_sb[0:1, ds(r * LT, LT)],
                                     AF.Exp, bias=c40[0:1, 0:1], scale=1.0,
                                     accum_out=ssum[0:1, ds(r, 1)])
                nc.vector.reciprocal(rcp[0:1, ds(r, 1)], ssum[0:1, ds(r, 1)])
                nc.vector.tensor_scalar_mul(attn[0:1, ds(r * LT, LT)],
                                            exps[0:1, ds(r * LT, LT)],
                                            rcp[0:1, ds(r, 1)])
                nc.sync.dma_start(out_ext[r, :], attn[0:1, ds(r * LT, LT)])

    nc.compile()
    _cached[key] = nc
    return nc


def _to_dev_dtype(a):
    if USE_FP8:
        return np.clip(a, -240.0, 240.0).astype(F8NP)
    return a.astype(BF)


def kernel(hidden, encoder_outputs, mask, W1, b1, W2, b2):
    global LAST_RESULT

    mask = np.asarray(mask, dtype=bool)
    idx = [np.nonzero(~mask[b])[0] for b in range(B)]
    cnt = np.array([len(i) for i in idx])
    main, rem = _geometry(int(cnt.max()))
    LT = main + sum(rem)
    nc = _build(main, rem)

    enc = np.asarray(encoder_outputs, dtype=np.float32)
    enc_t = np.transpose(enc, (1, 2, 0))            # [B, Hin, S]
    W1 = np.asarray(W1, dtype=np.float32)
    w1e = W1[:, :HIN].T                              # [Hin, H]
    w1h = W1[:, HIN:]                                # [H, H]
    hb = (np.asarray(hidden, np.float32) @ w1h.T
          + np.asarray(b1, np.float32).reshape(1, H))  # [B, H]
    w2 = np.asarray(W2, dtype=np.float32).reshape(H)

    # W1_enc.T packed for DoubleRow: [p, ht, it2, two, m]
    w1s = (w1e * W_SCALE) if USE_FP8 else w1e
    w1p = _to_dev_dtype(w1s).reshape(IT2, 2, P, HT, P)
    w1p = np.ascontiguousarray(np.transpose(w1p, (2, 3, 0, 1, 4))).reshape(P, -1)

    w2pad = np.zeros((P, HT, P), dtype=BF)
    w2pad[:, :, 0] = w2.reshape(HT, P).T
    w2pad = w2pad.reshape(P, HT * P)

    # packed enc per row + padneg
    encs = enc_t * ENC_SCALE if USE_FP8 else enc_t
    encm = np.zeros((B, HIN, main), dtype=F8NP if USE_FP8 else BF)
    encr = [np.zeros((N_CORES, HIN, BL * rk), dtype=F8NP if USE_FP8 else BF)
            for rk in rem]
    pneg = np.full((B, LT), np.float32(-1e30), dtype=np.float32)
    for b in range(B):
        c, rloc = divmod(b, BL)
        cols = _to_dev_dtype(encs[b][:, idx[b]])
        n = cnt[b]
        nm = min(n, main)
        encm[b, :, :nm] = cols[:, :nm]
        pneg[b, :n] = 0.0
        off = main
        for k, rk in enumerate(rem):
            if n > off:
                w = min(n - off, rk)
                encr[k][c, :, rloc * rk:rloc * rk + w] = cols[:, off:off + w]
            off += rk

    biasT = np.ascontiguousarray(
        np.transpose(hb.reshape(N_CORES, BL, HT, P), (0, 3, 2, 1))
    ).reshape(N_CORES, P, HT * BL).astype(np.float32)

    in_maps = []
    for c in range(N_CORES):
        sl = slice(c * BL, (c + 1) * BL)
        m = {
            "encm": np.ascontiguousarray(encm[sl]),
            "w1p": w1p,
            "biasT": biasT[c],
            "w2pad": w2pad,
            "padneg": np.ascontiguousarray(pneg[sl].reshape(-1)),
        }
        for k in range(len(rem)):
            m[f"encr{k}"] = np.ascontiguousarray(encr[k][c])
        in_maps.append(m)

    res = run_bass_kernel_spmd(nc, in_maps, core_ids=list(range(N_CORES)))
    LAST_RESULT = res

    out = np.zeros((B, S), dtype=np.float32)
    for b in range(B):
        c, rloc = divmod(b, BL)
        row = res.results[c]["out"][rloc]
        out[b, idx[b]] = row[:cnt[b]]
    return np.ascontiguousarray(out[:, None, :])


# revision 6
# speedup vs baseline: 2.4894x; 1.0855x over previous
"""Trainium2 Bass kernel for the attention-scoring MLP (nn_Attn):

    enc = encoder_outputs.transpose(1,0,2)          # [B,S,Hin]
    a1  = tanh(enc @ W1_enc.T + hidden @ W1_hid.T + b1)
    s   = a1 @ W2[0] (+ b2 -- dropped: softmax shift-invariant)
    s   = where(mask, -inf, s)
    out = softmax(s, axis=-1)[:, None, :]           # [B,1,S]

Strategy (v3):
  * Data-parallel over batch B=32 across 8 NeuronCores (4 rows each),
    weights replicated, no collectives.
  * Mask packing: masked positions get attn == 0 exactly, so only the
    ~50% unmasked columns of enc are shipped/computed. Host packs each
    row's unmasked columns; device computes scores+softmax on the packed
    stream; host scatters back to [B,1,S] with zeros. Geometry: each row
    contributes a MAIN-wide stream (<=448) plus its overflow columns;
    the 4 rows' overflows are batched into shared overflow streams so
    every matmul keeps free-dim >= 256 (tiny-FD matmuls are LDWEIGHTS-
    bound and waste the PE).
  * fp8 (e4m3) DoubleRow matmuls for the big enc @ W1_enc.T contraction:
    two 128-deep k-slices per instruction. Inputs are scaled host-side
    (enc x32, W1 x2^13) to clear fp8 subnormals; the 2^-18 compensation
    rides the tanh activation's scale port. The hidden @ W1_hid.T + b1
    term (0.1% of FLOPs) is folded host-side into the per-(h,b) tanh
    bias: row streams use the activation's per-partition bias port,
    overflow streams (mixed rows per tile) get the bias added on the
    otherwise-idle DVE, pre-scaled by 2^18 so the activation scale
    still matches.
  * W2 contraction: PSUM-accumulated bf16 matmul over h-tiles on the
    tanh output (padded [128,128] stationary, row 0 of PSUM used).
  * Softmax per row on-device: exp(s-40) with accumulate, reciprocal,
    scale, DMA out the packed attn row.
  * DMAs are batched (one per enc stream / weight tensor) to keep the
    instruction+semaphore count low -- the framework pre/postamble
    scales with it.
"""

import numpy as np
import ml_dtypes

import concourse.bass as bass
import concourse.tile as tile
from concourse import bacc, mybir
from concourse.bass import ds
from concourse.bass_utils import run_bass_kernel_spmd

N_CORES = 8
B, S, HIN, H = 32, 1024, 1024, 1024
BL = B // N_CORES          # local batch rows per core
P = 128                    # partitions
IT = HIN // P              # 128-deep contraction tiles
IT2 = IT // 2              # DoubleRow pair tiles
HT = H // P                # output-feature tiles
F32 = mybir.dt.float32
BF16 = mybir.dt.bfloat16
F8 = mybir.dt.float8e4
AF = mybir.ActivationFunctionType
DR = mybir.MatmulPerfMode.DoubleRow
BF = ml_dtypes.bfloat16
F8NP = ml_dtypes.float8_e4m3

USE_FP8 = True
ENC_SCALE = 32.0
W_SCALE = float(2.0 ** 13)
ACT_SCALE = float(1.0 / (ENC_SCALE * W_SCALE)) if USE_FP8 else 1.0
DELAY = 3                  # pending tanh->scores matmul pipeline depth

_cached = {}               # (main, rem) -> compiled Bacc
LAST_RESULT = None         # BassKernelResults of the most recent run


def _geometry(lmax: int):
    """MAIN width per row plus overflow-block widths (multiples of 16).

    MAIN is capped at 448 so a single overflow block (4 rows x <=128)
    covers typical masks with free-dim >= 256; longer rows fall back to
    extra overflow blocks.
    """
    rnd = lambda x: ((x + 15) // 16) * 16
    if lmax <= 448:
        return rnd(lmax), ()
    rem = []
    off = 448
    while off < lmax:
        rem.append(min(128, rnd(lmax - off)))
        off += rem[-1]
    return 448, tuple(rem)


def _build(main: int, rem: tuple):
    key = (main, rem)
    if key in _cached:
        return _cached[key]

    LT = main + sum(rem)
    EDT = F8 if USE_FP8 else BF16

    nc = bacc.Bacc("TRN2", target_bir_lowering=False, debug=False,
                   num_devices=N_CORES)

    # enc main stream per row: [b, hin, main]
    encm_ext = nc.dram_tensor("encm", [BL, HIN, main], EDT, kind="ExternalInput").ap()
    # shared overflow streams: [hin, BL*rem_k]
    encr_ext = [
        nc.dram_tensor(f"encr{k}", [HIN, BL * rk], EDT, kind="ExternalInput").ap()
        for k, rk in enumerate(rem)
    ]
    # W1_enc.T packed for DoubleRow: [p, ht, it2, two, m] flattened
    w1_ext = nc.dram_tensor("w1p", [P, HT * IT * P], EDT, kind="ExternalInput").ap()
    # per-(h,b) tanh bias = (hidden @ W1_hid.T + b1).T: [p, ht*BL]
    bias_ext = nc.dram_tensor("biasT", [P, HT * BL], F32, kind="ExternalInput").ap()
    # overflow-stream bias, pre-scaled by 1/ACT_SCALE, broadcast over cols
    biasr_ext = [
        nc.dram_tensor(f"biasR{k}", [P, HT * BL * rk], F32, kind="ExternalInput").ap()
        for k, rk in enumerate(rem)
    ]
    # W2 padded stationary: [p, ht*128], column 0 of each ht block = w2 chunk
    w2_ext = nc.dram_tensor("w2pad", [P, HT * P], BF16, kind="ExternalInput").ap()
    # 0 for real columns, -1e30 for padding: [BL*LT]
    pneg_ext = nc.dram_tensor("padneg", [BL * LT], F32, kind="ExternalInput").ap()
    out_ext = nc.dram_tensor("out", [BL, LT], F32, kind="ExternalOutput").ap()

    def wpair(w_sb, ht, it2):
        """Stationary [128, 2, 128] DoubleRow AP for (ht, it2)."""
        return w_sb[:, ds((ht * IT2 + it2) * 2 * P, 2 * P)].rearrange(
            "p (two m) -> p two m", two=2)

    with tile.TileContext(nc) as tc:
        with (
            tc.tile_pool(name="consts", bufs=1) as consts,
            tc.tile_pool(name="thp", bufs=6) as thp,
            tc.tile_pool(name="pap", bufs=2, space="PSUM") as pap,
            tc.tile_pool(name="pscp", bufs=2, space="PSUM") as pscp,
            tc.tile_pool(name="psrp", bufs=1, space="PSUM") as psrp,
        ):
            # ---- PE warmup: junk matmuls with no DMA deps so the HAM
            # clock-gate ramps while the first DMAs land.
            warm_sb = consts.tile([P, 512], BF16)
            nc.gpsimd.memset(warm_sb[:], 0.0)
            warm_ps = psrp.tile([P, 512], F32, tag="warm")
            for _ in range(8):
                nc.tensor.matmul(warm_ps[:], warm_sb[:, 0:P], warm_sb[:],
                                 start=True, stop=True)

            # ---- resident weights/constants; one batched DMA each.
            # Emission order = ring service order: first-needed first.
            w1_sb = consts.tile([P, HT * IT * P], EDT)
            nc.sync.dma_start(w1_sb[:], w1_ext[:, :])
            bias_sb = consts.tile([P, HT * BL], F32)
            nc.sync.dma_start(bias_sb[:], bias_ext[:, :])
            w2_sb = consts.tile([P, HT * P], BF16)
            nc.sync.dma_start(w2_sb[:], w2_ext[:, :])
            # overflow streams + their bias ride the scalar ring (parallel
            # with the weight wave on sync)
            encr_sb, biasr_sb = [], []
            for k, rk in enumerate(rem):
                w = BL * rk
                e = consts.tile([P, IT, w], EDT, tag=f"encr{k}")
                nc.scalar.dma_start(
                    e[:], encr_ext[k].rearrange("(it p) n -> p it n", p=P))
                encr_sb.append(e)
                bb = consts.tile([P, HT * w], F32, tag=f"biasR{k}")
                nc.scalar.dma_start(bb[:], biasr_ext[k][:, :])
                biasr_sb.append(bb)
            pneg_sb = consts.tile([1, BL * LT], F32)
            nc.scalar.dma_start(pneg_sb[:], pneg_ext[:])
            encm_sb = []
            for r in range(BL):
                e = consts.tile([P, IT, main], EDT, tag=f"encm{r}")
                nc.sync.dma_start(
                    e[:], encm_ext[r].rearrange("(it p) n -> p it n", p=P))
                encm_sb.append(e)

            scores_sb = consts.tile([1, BL * LT], F32)
            c40 = consts.tile([1, 1], F32)
            nc.gpsimd.memset(c40[:], -40.0)
            exps = consts.tile([1, BL * LT], F32)
            ssum = consts.tile([1, BL], F32)
            rcp = consts.tile([1, BL], F32)
            attn = consts.tile([1, BL * LT], F32)

            # ---- overflow streams first: every row's full score segment
            # is ready before that row's main-stream softmax tail runs.
            off = main
            for k, rk in enumerate(rem):
                w = BL * rk
                psr = psrp.tile([P, w], F32, tag=f"psr{k}")
                pend = []
                for ht in range(HT):
                    par = pap.tile([P, w], F32, tag="par")
                    for it2 in range(IT2):
                        if USE_FP8:
                            nc.tensor.matmul(
                                par[:], wpair(w1_sb, ht, it2),
                                encr_sb[k][:, ds(2 * it2, 2), :],
                                start=(it2 == 0), stop=(it2 == IT2 - 1),
                                perf_mode=DR)
                        else:
                            for i in range(2):
                                nc.tensor.matmul(
                                    par[:],
                                    w1_sb[:, ds(((ht * IT2 + it2) * 2 + i) * P, P)],
                                    encr_sb[k][:, ds(2 * it2 + i, 1), :],
                                    start=(it2 == 0 and i == 0),
                                    stop=(it2 == IT2 - 1 and i == 1))
                    # bias (pre-scaled) on DVE; tanh on ACT
                    tp = thp.tile([P, w], F32, tag="tpre")
                    nc.vector.tensor_add(tp[:], par[:],
                                         biasr_sb[k][:, ds(ht * w, w)])
                    th = thp.tile([P, w], BF16, tag="thr")
                    nc.scalar.activation(th[:], tp[:], AF.Tanh,
                                         scale=ACT_SCALE)
                    pend.append((th, ht))
                    if len(pend) > 2:
                        pth, pht = pend.pop(0)
                        nc.tensor.matmul(psr[:], w2_sb[:, ds(pht * P, P)],
                                         pth[:], start=(pht == 0),
                                         stop=(pht == HT - 1))
                for pth, pht in pend:
                    nc.tensor.matmul(psr[:], w2_sb[:, ds(pht * P, P)], pth[:],
                                     start=(pht == 0), stop=(pht == HT - 1))
                for r in range(BL):
                    pos = r * LT + off
                    nc.vector.tensor_add(scores_sb[0:1, ds(pos, rk)],
                                         psr[0:1, ds(r * rk, rk)],
                                         pneg_sb[0:1, ds(pos, rk)])
                off += rk

            # ---- main streams: per batch row.
            for r in range(BL):
                psc = pscp.tile([P, main], F32, tag="psc")
                pend = []
                for ht in range(HT):
                    pa = pap.tile([P, main], F32, tag="pa")
                    for it2 in range(IT2):
                        if USE_FP8:
                            nc.tensor.matmul(
                                pa[:], wpair(w1_sb, ht, it2),
                                encm_sb[r][:, ds(2 * it2, 2), :],
                                start=(it2 == 0), stop=(it2 == IT2 - 1),
                                perf_mode=DR)
                        else:
                            for i in range(2):
                                nc.tensor.matmul(
                                    pa[:],
                                    w1_sb[:, ds(((ht * IT2 + it2) * 2 + i) * P, P)],
                                    encm_sb[r][:, ds(2 * it2 + i, 1), :],
                                    start=(it2 == 0 and i == 0),
                                    stop=(it2 == IT2 - 1 and i == 1))
                    th = thp.tile([P, main], BF16, tag="th")
                    nc.scalar.activation(th[:], pa[:], AF.Tanh,
                                         bias=bias_sb[:, ds(ht * BL + r, 1)],
                                         scale=ACT_SCALE)
                    pend.append((th, ht))
                    if len(pend) > DELAY:
                        pth, pht = pend.pop(0)
                        nc.tensor.matmul(psc[:], w2_sb[:, ds(pht * P, P)],
                                         pth[:], start=(pht == 0),
                                         stop=(pht == HT - 1))
                for pth, pht in pend:
                    nc.tensor.matmul(psc[:], w2_sb[:, ds(pht * P, P)], pth[:],
                                     start=(pht == 0), stop=(pht == HT - 1))

                # ---- softmax tail for row r (overflow scores already in).
                nc.vector.tensor_add(scores_sb[0:1, ds(r * LT, main)],
                                     psc[0:1, :],
                                     pneg_sb[0:1, ds(r * LT, main)])
                # |scores| <= ||w2||_1 <= 16, so exp(s - 40) never overflows
                # and softmax is shift-invariant -- no max-reduce needed.
                nc.scalar.activation(exps[0:1, ds(r * LT, LT)],
                                     scores_sb[0:1, ds(r * LT, LT)],
                                     AF.Exp, bias=c40[0:1, 0:1], scale=1.0,
                                     accum_out=ssum[0:1, ds(r, 1)])
                nc.vector.reciprocal(rcp[0:1, ds(r, 1)], ssum[0:1, ds(r, 1)])
                nc.vector.tensor_scalar_mul(attn[0:1, ds(r * LT, LT)],
                                            exps[0:1, ds(r * LT, LT)],
                                            rcp[0:1, ds(r, 1)])
                nc.scalar.dma_start(out_ext[r, :], attn[0:1, ds(r * LT, LT)])

    nc.compile()
    _cached[key] = nc
    return nc


def _to_dev_dtype(a):
    if USE_FP8:
        return np.clip(a, -240.0, 240.0).astype(F8NP)
    return a.astype(BF)


def kernel(hidden, encoder_outputs, mask, W1, b1, W2, b2):
    global LAST_RESULT

    mask = np.asarray(mask, dtype=bool)
    idx = [np.nonzero(~mask[b])[0] for b in range(B)]
    cnt = np.array([len(i) for i in idx])
    main, rem = _geometry(int(cnt.max()))
    LT = main + sum(rem)
    nc = _build(main, rem)

    enc = np.asarray(encoder_outputs, dtype=np.float32)
    enc_t = np.transpose(enc, (1, 2, 0))            # [B, Hin, S]
    W1 = np.asarray(W1, dtype=np.float32)
    w1e = W1[:, :HIN].T                              # [Hin, H]
    w1h = W1[:, HIN:]                                # [H, H]
    hb = (np.asarray(hidden, np.float32) @ w1h.T
          + np.asarray(b1, np.float32).reshape(1, H))  # [B, H]
    w2 = np.asarray(W2, dtype=np.float32).reshape(H)

    # W1_enc.T packed for DoubleRow: [p, ht, it2, two, m]
    w1s = (w1e * W_SCALE) if USE_FP8 else w1e
    w1p = _to_dev_dtype(w1s).reshape(IT2, 2, P, HT, P)
    w1p = np.ascontiguousarray(np.transpose(w1p, (2, 3, 0, 1, 4))).reshape(P, -1)

    w2pad = np.zeros((P, HT, P), dtype=BF)
    w2pad[:, :, 0] = w2.reshape(HT, P).T
    w2pad = w2pad.reshape(P, HT * P)

    # packed enc per row + padneg
    encs = enc_t * ENC_SCALE if USE_FP8 else enc_t
    edt = F8NP if USE_FP8 else BF
    encm = np.zeros((B, HIN, main), dtype=edt)
    encr = [np.zeros((N_CORES, HIN, BL * rk), dtype=edt) for rk in rem]
    pneg = np.full((B, LT), np.float32(-1e30), dtype=np.float32)
    for b in range(B):
        c, rloc = divmod(b, BL)
        cols = _to_dev_dtype(encs[b][:, idx[b]])
        n = cnt[b]
        nm = min(n, main)
        encm[b, :, :nm] = cols[:, :nm]
        pneg[b, :n] = 0.0
        off = main
        for k, rk in enumerate(rem):
            if n > off:
                w = min(n - off, rk)
                encr[k][c, :, rloc * rk:rloc * rk + w] = cols[:, off:off + w]
            off += rk

    # [c, p, ht*BL] per-(h,b) bias
    biasT = np.ascontiguousarray(
        np.transpose(hb.reshape(N_CORES, BL, HT, P), (0, 3, 2, 1))
    ).reshape(N_CORES, P, HT * BL).astype(np.float32)
    # overflow bias: [c, p, ht*(BL*rk)] broadcast over cols, pre-scaled
    biasR = []
    for k, rk in enumerate(rem):
        bb = np.transpose(hb.reshape(N_CORES, BL, HT, P), (0, 3, 2, 1))
        bb = np.repeat(bb[:, :, :, :, None], rk, axis=4)   # [c,p,ht,BL,rk]
        biasR.append(np.ascontiguousarray(
            bb.reshape(N_CORES, P, HT * BL * rk) / ACT_SCALE
        ).astype(np.float32))

    in_maps = []
    for c in range(N_CORES):
        sl = slice(c * BL, (c + 1) * BL)
        m = {
            "encm": np.ascontiguousarray(encm[sl]),
            "w1p": w1p,
            "biasT": biasT[c],
            "w2pad": w2pad,
            "padneg": np.ascontiguousarray(pneg[sl].reshape(-1)),
        }
        for k in range(len(rem)):
            m[f"encr{k}"] = np.ascontiguousarray(encr[k][c])
            m[f"biasR{k}"] = biasR[k][c]
        in_maps.append(m)

    res = run_bass_kernel_spmd(nc, in_maps, core_ids=list(range(N_CORES)))
    LAST_RESULT = res

    out = np.zeros((B, S), dtype=np.float32)
    for b in range(B):
        c, rloc = divmod(b, BL)
        row = res.results[c]["out"][rloc]
        out[b, idx[b]] = row[:cnt[b]]
    return np.ascontiguousarray(out[:, None, :])


# revision 14
# speedup vs baseline: 2.5826x; 1.0374x over previous
"""Trainium2 Bass kernel for the attention-scoring MLP (nn_Attn):

    enc = encoder_outputs.transpose(1,0,2)          # [B,S,Hin]
    a1  = tanh(enc @ W1_enc.T + hidden @ W1_hid.T + b1)
    s   = a1 @ W2[0] (+ b2 -- dropped: softmax shift-invariant)
    s   = where(mask, -inf, s)
    out = softmax(s, axis=-1)[:, None, :]           # [B,1,S]

Strategy (v3):
  * Data-parallel over batch B=32 across 8 NeuronCores (4 rows each),
    weights replicated, no collectives.
  * Mask packing: masked positions get attn == 0 exactly, so only the
    ~50% unmasked columns of enc are shipped/computed. Host packs each
    row's unmasked columns; device computes scores+softmax on the packed
    stream; host scatters back to [B,1,S] with zeros. Geometry: each row
    contributes a MAIN-wide stream (<=448) plus its overflow columns;
    the 4 rows' overflows are batched into shared overflow streams so
    every matmul keeps free-dim >= 256 (tiny-FD matmuls are LDWEIGHTS-
    bound and waste the PE).
  * fp8 (e4m3) DoubleRow matmuls for the big enc @ W1_enc.T contraction:
    two 128-deep k-slices per instruction. Inputs are scaled host-side
    (enc x32, W1 x2^13) to clear fp8 subnormals; the 2^-18 compensation
    rides the tanh activation's scale port. The hidden @ W1_hid.T + b1
    term (0.1% of FLOPs) is folded host-side into the per-(h,b) tanh
    bias: row streams use the activation's per-partition bias port,
    overflow streams (mixed rows per tile) get the bias added on the
    otherwise-idle DVE, pre-scaled by 2^18 so the activation scale
    still matches.
  * W2 contraction: PSUM-accumulated bf16 matmul over h-tiles on the
    tanh output (padded [128,128] stationary, row 0 of PSUM used).
  * Softmax per row on-device: exp(s-40) with accumulate, reciprocal,
    scale, DMA out the packed attn row.
  * DMAs are batched (one per enc stream / weight tensor) to keep the
    instruction+semaphore count low -- the framework pre/postamble
    scales with it.
"""

import numpy as np
import ml_dtypes

import concourse.bass as bass
import concourse.tile as tile
from concourse import bacc, mybir
from concourse.bass import ds
from concourse.bass_utils import run_bass_kernel_spmd

N_CORES = 8
B, S, HIN, H = 32, 1024, 1024, 1024
BL = B // N_CORES          # local batch rows per core
P = 128                    # partitions
IT = HIN // P              # 128-deep contraction tiles
IT2 = IT // 2              # DoubleRow pair tiles
HT = H // P                # output-feature tiles
F32 = mybir.dt.float32
BF16 = mybir.dt.bfloat16
F8 = mybir.dt.float8e4
AF = mybir.ActivationFunctionType
DR = mybir.MatmulPerfMode.DoubleRow
BF = ml_dtypes.bfloat16
F8NP = ml_dtypes.float8_e4m3

USE_FP8 = True
ENC_SCALE = 32.0
W_SCALE = float(2.0 ** 13)
ACT_SCALE = float(1.0 / (ENC_SCALE * W_SCALE)) if USE_FP8 else 1.0
DELAY = 3                  # pending tanh->scores matmul pipeline depth

_cached = {}               # (main, rem) -> compiled Bacc
LAST_RESULT = None         # BassKernelResults of the most recent run


def _geometry(lmax: int):
    """MAIN width per row plus overflow-block widths (multiples of 16).

    MAIN is capped at 448 so a single overflow block (4 rows x <=128)
    covers typical masks with free-dim >= 256; longer rows fall back to
    extra overflow blocks.
    """
    rnd = lambda x: ((x + 15) // 16) * 16
    if lmax <= 448:
        return rnd(lmax), ()
    rem = []
    off = 448
    while off < lmax:
        rem.append(min(128, rnd(lmax - off)))
        off += rem[-1]
    return 448, tuple(rem)


def _build(main: int, rem: tuple):
    key = (main, rem)
    if key in _cached:
        return _cached[key]

    LT = main + sum(rem)
    EDT = F8 if USE_FP8 else BF16

    nc = bacc.Bacc("TRN2", target_bir_lowering=False, debug=False,
                   num_devices=N_CORES)

    # enc main stream per row: [b, hin, main]
    encm_ext = nc.dram_tensor("encm", [BL, HIN, main], EDT, kind="ExternalInput").ap()
    # shared overflow streams: [hin, BL*rem_k]
    encr_ext = [
        nc.dram_tensor(f"encr{k}", [HIN, BL * rk], EDT, kind="ExternalInput").ap()
        for k, rk in enumerate(rem)
    ]
    # W1_enc.T packed for DoubleRow: [p, ht, it2, two, m] flattened
    w1_ext = nc.dram_tensor("w1p", [P, HT * IT * P], EDT, kind="ExternalInput").ap()
    # per-(h,b) tanh bias = (hidden @ W1_hid.T + b1).T: [p, ht*BL]
    bias_ext = nc.dram_tensor("biasT", [P, HT * BL], F32, kind="ExternalInput").ap()
    # same, pre-scaled by 1/ACT_SCALE (overflow streams add it on DVE
    # before the activation's scale is applied)
    bias2_ext = nc.dram_tensor("bias2", [P, HT * BL], F32, kind="ExternalInput").ap()
    # W2 padded stationary: [p, ht*128], column 0 of each ht block = w2 chunk
    w2_ext = nc.dram_tensor("w2pad", [P, HT * P], BF16, kind="ExternalInput").ap()
    # 0 for real columns, -1e30 for padding: [BL*LT]
    pneg_ext = nc.dram_tensor("padneg", [BL * LT], F32, kind="ExternalInput").ap()
    out_ext = nc.dram_tensor("out", [BL, LT], F32, kind="ExternalOutput").ap()

    def wpair(w_sb, ht, it2):
        """Stationary [128, 2, 128] DoubleRow AP for (ht, it2)."""
        return w_sb[:, ds((ht * IT2 + it2) * 2 * P, 2 * P)].rearrange(
            "p (two m) -> p two m", two=2)

    with tile.TileContext(nc) as tc:
        with (
            tc.tile_pool(name="consts", bufs=1) as consts,
            tc.tile_pool(name="thp", bufs=6) as thp,
            tc.tile_pool(name="pap", bufs=3, space="PSUM") as pap,
            tc.tile_pool(name="parp", bufs=2, space="PSUM") as parp,
            tc.tile_pool(name="pscp", bufs=2, space="PSUM") as pscp,
            tc.tile_pool(name="psrp", bufs=1, space="PSUM") as psrp,
        ):
            # ---- PE warmup: junk matmuls with no DMA deps so the HAM
            # clock-gate ramps while the first DMAs land.
            warm_sb = consts.tile([P, 512], BF16)
            nc.gpsimd.memset(warm_sb[:], 0.0)
            for _ in range(8):
                warm_ps = pap.tile([P, main], F32, tag="pa")
                nc.tensor.matmul(warm_ps[:], warm_sb[:, 0:P],
                                 warm_sb[:, 0:main], start=True, stop=True)

            # ---- resident weights/constants.
            # Emission order = ring service order: first-needed first.
            # w1 split per ht so the first overflow matmul only waits for
            # one eighth of the 1MB weight load.
            w1_sb = consts.tile([P, HT * IT * P], EDT)
            for ht in range(HT):
                nc.sync.dma_start(w1_sb[:, ds(ht * IT * P, IT * P)],
                                  w1_ext[:, ds(ht * IT * P, IT * P)])
            # overflow streams + biases ride the scalar ring (parallel
            # with the weight wave on sync)
            encr_sb = []
            for k, rk in enumerate(rem):
                w = BL * rk
                e = consts.tile([P, IT, w], EDT, tag=f"encr{k}")
                nc.scalar.dma_start(
                    e[:], encr_ext[k].rearrange("(it p) n -> p it n", p=P))
                encr_sb.append(e)
            bias2_sb = consts.tile([P, HT * BL], F32)
            nc.scalar.dma_start(bias2_sb[:], bias2_ext[:, :])
            w2_sb = consts.tile([P, HT * P], BF16)
            nc.sync.dma_start(w2_sb[:], w2_ext[:, :])
            pneg_sb = consts.tile([1, BL * LT], F32)
            nc.scalar.dma_start(pneg_sb[:], pneg_ext[:])
            bias_sb = consts.tile([P, HT * BL], F32)
            nc.scalar.dma_start(bias_sb[:], bias_ext[:, :])
            encm_sb = []
            for r in range(BL):
                e = consts.tile([P, IT, main], EDT, tag=f"encm{r}")
                eng = nc.scalar if r < 2 else nc.sync
                eng.dma_start(
                    e[:], encm_ext[r].rearrange("(it p) n -> p it n", p=P))
                encm_sb.append(e)

            scores_sb = consts.tile([1, BL * LT], F32)
            c40 = consts.tile([1, 1], F32)
            nc.gpsimd.memset(c40[:], -40.0)
            exps = consts.tile([1, BL * LT], F32)
            ssum = consts.tile([1, BL], F32)
            rcp = consts.tile([1, BL], F32)
            attn = consts.tile([1, BL * LT], F32)

            # ---- overflow streams first: every row's full score segment
            # is ready before that row's main-stream softmax tail runs.
            off = main
            for k, rk in enumerate(rem):
                w = BL * rk
                psr = psrp.tile([P, w], F32, tag=f"psr{k}")
                pend = []
                for ht in range(HT):
                    par = parp.tile([P, w], F32, tag="par")
                    for it2 in range(IT2):
                        if USE_FP8:
                            nc.tensor.matmul(
                                par[:], wpair(w1_sb, ht, it2),
                                encr_sb[k][:, ds(2 * it2, 2), :],
                                start=(it2 == 0), stop=(it2 == IT2 - 1),
                                perf_mode=DR)
                        else:
                            for i in range(2):
                                nc.tensor.matmul(
                                    par[:],
                                    w1_sb[:, ds(((ht * IT2 + it2) * 2 + i) * P, P)],
                                    encr_sb[k][:, ds(2 * it2 + i, 1), :],
                                    start=(it2 == 0 and i == 0),
                                    stop=(it2 == IT2 - 1 and i == 1))
                    # per-row bias (pre-scaled) on DVE; tanh on ACT
                    tp = thp.tile([P, w], F32, tag="tpre")
                    for r in range(BL):
                        nc.vector.tensor_scalar_add(
                            tp[:, ds(r * rk, rk)], par[:, ds(r * rk, rk)],
                            bias2_sb[:, ds(ht * BL + r, 1)])
                    th = thp.tile([P, w], BF16, tag="thr")
                    nc.scalar.activation(th[:], tp[:], AF.Tanh,
                                         scale=ACT_SCALE)
                    pend.append((th, ht))
                    if len(pend) > 2:
                        pth, pht = pend.pop(0)
                        nc.tensor.matmul(psr[:], w2_sb[:, ds(pht * P, P)],
                                         pth[:], start=(pht == 0),
                                         stop=(pht == HT - 1))
                for pth, pht in pend:
                    nc.tensor.matmul(psr[:], w2_sb[:, ds(pht * P, P)], pth[:],
                                     start=(pht == 0), stop=(pht == HT - 1))
                for r in range(BL):
                    pos = r * LT + off
                    nc.vector.tensor_add(scores_sb[0:1, ds(pos, rk)],
                                         psr[0:1, ds(r * rk, rk)],
                                         pneg_sb[0:1, ds(pos, rk)])
                off += rk

            # ---- main streams: per batch row, with a single pending-
            # scores queue across rows so row-end drains interleave with
            # the next row's matmuls instead of bubbling the PE.
            def tail(r, psc):
                # softmax tail for row r (overflow scores already in).
                nc.vector.tensor_add(scores_sb[0:1, ds(r * LT, main)],
                                     psc[0:1, :],
                                     pneg_sb[0:1, ds(r * LT, main)])
                # |scores| <= ||w2||_1 <= 16, so exp(s - 40) never overflows
                # and softmax is shift-invariant -- no max-reduce needed.
                nc.scalar.activation(exps[0:1, ds(r * LT, LT)],
                                     scores_sb[0:1, ds(r * LT, LT)],
                                     AF.Exp, bias=c40[0:1, 0:1], scale=1.0,
                                     accum_out=ssum[0:1, ds(r, 1)])
                nc.vector.reciprocal(rcp[0:1, ds(r, 1)], ssum[0:1, ds(r, 1)])
                nc.vector.tensor_scalar_mul(attn[0:1, ds(r * LT, LT)],
                                            exps[0:1, ds(r * LT, LT)],
                                            rcp[0:1, ds(r, 1)])
                nc.sync.dma_start(out_ext[r, :], attn[0:1, ds(r * LT, LT)])

            pend = []

            def pop_scores():
                pth, pht, ppsc, prow = pend.pop(0)
                nc.tensor.matmul(ppsc[:], w2_sb[:, ds(pht * P, P)], pth[:],
                                 start=(pht == 0), stop=(pht == HT - 1))
                if pht == HT - 1:
                    tail(prow, ppsc)

            for r in range(BL):
                psc = pscp.tile([P, main], F32, tag="psc")
                for ht in range(HT):
                    pa = pap.tile([P, main], F32, tag="pa")
                    for it2 in range(IT2):
                        if USE_FP8:
                            nc.tensor.matmul(
                                pa[:], wpair(w1_sb, ht, it2),
                                encm_sb[r][:, ds(2 * it2, 2), :],
                                start=(it2 == 0), stop=(it2 == IT2 - 1),
                                perf_mode=DR)
                        else:
                            for i in range(2):
                                nc.tensor.matmul(
                                    pa[:],
                                    w1_sb[:, ds(((ht * IT2 + it2) * 2 + i) * P, P)],
                                    encm_sb[r][:, ds(2 * it2 + i, 1), :],
                                    start=(it2 == 0 and i == 0),
                                    stop=(it2 == IT2 - 1 and i == 1))
                    th = thp.tile([P, main], BF16, tag="th")
                    nc.scalar.activation(th[:], pa[:], AF.Tanh,
                                         bias=bias_sb[:, ds(ht * BL + r, 1)],
                                         scale=ACT_SCALE)
                    pend.append((th, ht, psc, r))
                    if len(pend) > DELAY:
                        pop_scores()
            while pend:
                pop_scores()

    nc.compile()
    _cached[key] = nc
    return nc


def _to_dev_dtype(a):
    if USE_FP8:
        return np.clip(a, -240.0, 240.0).astype(F8NP)
    return a.astype(BF)


def kernel(hidden, encoder_outputs, mask, W1, b1, W2, b2):
    global LAST_RESULT

    mask = np.asarray(mask, dtype=bool)
    idx = [np.nonzero(~mask[b])[0] for b in range(B)]
    cnt = np.array([len(i) for i in idx])
    main, rem = _geometry(int(cnt.max()))
    LT = main + sum(rem)
    nc = _build(main, rem)

    enc = np.asarray(encoder_outputs, dtype=np.float32)
    enc_t = np.transpose(enc, (1, 2, 0))            # [B, Hin, S]
    W1 = np.asarray(W1, dtype=np.float32)
    w1e = W1[:, :HIN].T                              # [Hin, H]
    w1h = W1[:, HIN:]                                # [H, H]
    hb = (np.asarray(hidden, np.float32) @ w1h.T
          + np.asarray(b1, np.float32).reshape(1, H))  # [B, H]
    w2 = np.asarray(W2, dtype=np.float32).reshape(H)

    # W1_enc.T packed for DoubleRow: [p, ht, it2, two, m]
    w1s = (w1e * W_SCALE) if USE_FP8 else w1e
    w1p = _to_dev_dtype(w1s).reshape(IT2, 2, P, HT, P)
    w1p = np.ascontiguousarray(np.transpose(w1p, (2, 3, 0, 1, 4))).reshape(P, -1)

    w2pad = np.zeros((P, HT, P), dtype=BF)
    w2pad[:, :, 0] = w2.reshape(HT, P).T
    w2pad = w2pad.reshape(P, HT * P)

    # packed enc per row + padneg
    encs = enc_t * ENC_SCALE if USE_FP8 else enc_t
    edt = F8NP if USE_FP8 else BF
    encm = np.zeros((B, HIN, main), dtype=edt)
    encr = [np.zeros((N_CORES, HIN, BL * rk), dtype=edt) for rk in rem]
    pneg = np.full((B, LT), np.float32(-1e30), dtype=np.float32)
    for b in range(B):
        c, rloc = divmod(b, BL)
        cols = _to_dev_dtype(encs[b][:, idx[b]])
        n = cnt[b]
        nm = min(n, main)
        encm[b, :, :nm] = cols[:, :nm]
        pneg[b, :n] = 0.0
        off = main
        for k, rk in enumerate(rem):
            if n > off:
                w = min(n - off, rk)
                encr[k][c, :, rloc * rk:rloc * rk + w] = cols[:, off:off + w]
            off += rk

    # [c, p, ht*BL] per-(h,b) bias
    biasT = np.ascontiguousarray(
        np.transpose(hb.reshape(N_CORES, BL, HT, P), (0, 3, 2, 1))
    ).reshape(N_CORES, P, HT * BL).astype(np.float32)
    bias2 = (biasT / ACT_SCALE).astype(np.float32)

    in_maps = []
    for c in range(N_CORES):
        sl = slice(c * BL, (c + 1) * BL)
        m = {
            "encm": np.ascontiguousarray(encm[sl]),
            "w1p": w1p,
            "biasT": biasT[c],
            "bias2": bias2[c],
            "w2pad": w2pad,
            "padneg": np.ascontiguousarray(pneg[sl].reshape(-1)),
        }
        for k in range(len(rem)):
            m[f"encr{k}"] = np.ascontiguousarray(encr[k][c])
        in_maps.append(m)

    res = run_bass_kernel_spmd(nc, in_maps, core_ids=list(range(N_CORES)))
    LAST_RESULT = res

    out = np.zeros((B, S), dtype=np.float32)
    for b in range(B):
        c, rloc = divmod(b, BL)
        row = res.results[c]["out"][rloc]
        out[b, idx[b]] = row[:cnt[b]]
    return np.ascontiguousarray(out[:, None, :])


# revision 18
# speedup vs baseline: 2.6401x; 1.0223x over previous
"""Trainium2 Bass kernel for the attention-scoring MLP (nn_Attn):

    enc = encoder_outputs.transpose(1,0,2)          # [B,S,Hin]
    a1  = tanh(enc @ W1_enc.T + hidden @ W1_hid.T + b1)
    s   = a1 @ W2[0] (+ b2 -- dropped: softmax shift-invariant)
    s   = where(mask, -inf, s)
    out = softmax(s, axis=-1)[:, None, :]           # [B,1,S]

Strategy (v3):
  * Data-parallel over batch B=32 across 8 NeuronCores (4 rows each),
    weights replicated, no collectives.
  * Mask packing: masked positions get attn == 0 exactly, so only the
    ~50% unmasked columns of enc are shipped/computed. Host packs each
    row's unmasked columns; device computes scores+softmax on the packed
    stream; host scatters back to [B,1,S] with zeros. Geometry: each row
    contributes a MAIN-wide stream (<=448) plus its overflow columns;
    the 4 rows' overflows are batched into shared overflow streams so
    every matmul keeps free-dim >= 256 (tiny-FD matmuls are LDWEIGHTS-
    bound and waste the PE).
  * fp8 (e4m3) DoubleRow matmuls for the big enc @ W1_enc.T contraction:
    two 128-deep k-slices per instruction. Inputs are scaled host-side
    (enc x32, W1 x2^13) to clear fp8 subnormals; the 2^-18 compensation
    rides the tanh activation's scale port. The hidden @ W1_hid.T + b1
    term (0.1% of FLOPs) is folded host-side into the per-(h,b) tanh
    bias: row streams use the activation's per-partition bias port,
    overflow streams (mixed rows per tile) get the bias added on the
    otherwise-idle DVE, pre-scaled by 2^18 so the activation scale
    still matches.
  * W2 contraction: PSUM-accumulated bf16 matmul over h-tiles on the
    tanh output (padded [128,128] stationary, row 0 of PSUM used).
  * Softmax per row on-device: exp(s-40) with accumulate, reciprocal,
    scale, DMA out the packed attn row.
  * DMAs are batched (one per enc stream / weight tensor) to keep the
    instruction+semaphore count low -- the framework pre/postamble
    scales with it.
"""

import numpy as np
import ml_dtypes

import concourse.bass as bass
import concourse.tile as tile
from concourse import bacc, mybir
from concourse.bass import ds
from concourse.bass_utils import run_bass_kernel_spmd

N_CORES = 8
B, S, HIN, H = 32, 1024, 1024, 1024
BL = B // N_CORES          # local batch rows per core
P = 128                    # partitions
IT = HIN // P              # 128-deep contraction tiles
IT2 = IT // 2              # DoubleRow pair tiles
HT = H // P                # output-feature tiles
F32 = mybir.dt.float32
BF16 = mybir.dt.bfloat16
F8 = mybir.dt.float8e4
AF = mybir.ActivationFunctionType
DR = mybir.MatmulPerfMode.DoubleRow
BF = ml_dtypes.bfloat16
F8NP = ml_dtypes.float8_e4m3

USE_FP8 = True
ENC_SCALE = 32.0
W_SCALE = float(2.0 ** 13)
ACT_SCALE = float(1.0 / (ENC_SCALE * W_SCALE)) if USE_FP8 else 1.0
DELAY = 3                  # pending tanh->scores matmul pipeline depth

_cached = {}               # (main, rem) -> compiled Bacc
LAST_RESULT = None         # BassKernelResults of the most recent run


def _geometry(lmax: int):
    """MAIN width per row plus overflow-block widths (multiples of 16).

    MAIN is capped at 448 so a single overflow block (4 rows x <=128)
    covers typical masks with free-dim >= 256; longer rows fall back to
    extra overflow blocks.
    """
    rnd = lambda x: ((x + 15) // 16) * 16
    if lmax <= 448:
        return rnd(lmax), ()
    rem = []
    off = 448
    while off < lmax:
        rem.append(min(128, rnd(lmax - off)))
        off += rem[-1]
    return 448, tuple(rem)


def _build(main: int, rem: tuple):
    key = (main, rem)
    if key in _cached:
        return _cached[key]

    LT = main + sum(rem)
    EDT = F8 if USE_FP8 else BF16

    nc = bacc.Bacc("TRN2", target_bir_lowering=False, debug=False,
                   num_devices=N_CORES)

    # enc main stream per row: [b, hin, main]
    encm_ext = nc.dram_tensor("encm", [BL, HIN, main], EDT, kind="ExternalInput").ap()
    # shared overflow streams: [hin, BL*rem_k]
    encr_ext = [
        nc.dram_tensor(f"encr{k}", [HIN, BL * rk], EDT, kind="ExternalInput").ap()
        for k, rk in enumerate(rem)
    ]
    # W1_enc.T packed for DoubleRow: [p, ht, it2, two, m] flattened
    w1_ext = nc.dram_tensor("w1p", [P, HT * IT * P], EDT, kind="ExternalInput").ap()
    # per-(h,b) tanh bias = (hidden @ W1_hid.T + b1).T: [p, ht*BL]
    bias_ext = nc.dram_tensor("biasT", [P, HT * BL], F32, kind="ExternalInput").ap()
    # same, pre-scaled by 1/ACT_SCALE (overflow streams add it on DVE
    # before the activation's scale is applied)
    bias2_ext = nc.dram_tensor("bias2", [P, HT * BL], F32, kind="ExternalInput").ap()
    # W2 padded stationary: [p, ht*128], column 0 of each ht block = w2 chunk
    w2_ext = nc.dram_tensor("w2pad", [P, HT * P], BF16, kind="ExternalInput").ap()
    # 0 for real columns, -1e30 for padding: [BL*LT]
    pneg_ext = nc.dram_tensor("padneg", [BL * LT], F32, kind="ExternalInput").ap()
    out_ext = nc.dram_tensor("out", [BL, LT], F32, kind="ExternalOutput").ap()

    def wpair(w_sb, ht, it2):
        """Stationary [128, 2, 128] DoubleRow AP for (ht, it2)."""
        return w_sb[:, ds((ht * IT2 + it2) * 2 * P, 2 * P)].rearrange(
            "p (two m) -> p two m", two=2)

    with tile.TileContext(nc) as tc:
        with (
            tc.tile_pool(name="consts", bufs=1) as consts,
            tc.tile_pool(name="thp", bufs=6) as thp,
            tc.tile_pool(name="pap", bufs=3, space="PSUM") as pap,
            tc.tile_pool(name="parp", bufs=2, space="PSUM") as parp,
            tc.tile_pool(name="pscp", bufs=2, space="PSUM") as pscp,
            tc.tile_pool(name="psrp", bufs=1, space="PSUM") as psrp,
        ):
            # ---- PE warmup: junk matmuls with no DMA deps so the HAM
            # clock-gate ramps while the first DMAs land.
            warm_sb = consts.tile([P, 512], BF16)
            nc.gpsimd.memset(warm_sb[:], 0.0)
            for _ in range(8):
                warm_ps = pap.tile([P, main], F32, tag="pa")
                nc.tensor.matmul(warm_ps[:], warm_sb[:, 0:P],
                                 warm_sb[:, 0:main], start=True, stop=True)

            # ---- resident weights/constants.
            # Emission order = ring service order: first-needed first.
            # w1 split per ht so the first overflow matmul only waits for
            # one eighth of the 1MB weight load.
            w1_sb = consts.tile([P, HT * IT * P], EDT)
            for ht in range(HT):
                nc.sync.dma_start(w1_sb[:, ds(ht * IT * P, IT * P)],
                                  w1_ext[:, ds(ht * IT * P, IT * P)])
            # overflow streams + biases ride the scalar ring (parallel
            # with the weight wave on sync)
            encr_sb = []
            for k, rk in enumerate(rem):
                w = BL * rk
                e = consts.tile([P, IT, w], EDT, tag=f"encr{k}")
                nc.scalar.dma_start(
                    e[:], encr_ext[k].rearrange("(it p) n -> p it n", p=P))
                encr_sb.append(e)
            bias2_sb = consts.tile([P, HT * BL], F32)
            nc.scalar.dma_start(bias2_sb[:], bias2_ext[:, :])
            w2_sb = consts.tile([P, HT * P], BF16)
            nc.sync.dma_start(w2_sb[:], w2_ext[:, :])
            pneg_sb = consts.tile([1, BL * LT], F32)
            nc.scalar.dma_start(pneg_sb[:], pneg_ext[:])
            bias_sb = consts.tile([P, HT * BL], F32)
            nc.scalar.dma_start(bias_sb[:], bias_ext[:, :])
            encm_sb = []
            for r in range(BL):
                e = consts.tile([P, IT, main], EDT, tag=f"encm{r}")
                eng = nc.scalar if r < 2 else nc.sync
                eng.dma_start(
                    e[:], encm_ext[r].rearrange("(it p) n -> p it n", p=P))
                encm_sb.append(e)

            scores_sb = consts.tile([1, BL * LT], F32)
            c40 = consts.tile([1, 1], F32)
            nc.gpsimd.memset(c40[:], -40.0)
            exps = consts.tile([1, BL * LT], F32)
            ssum = consts.tile([1, BL], F32)
            rcp = consts.tile([1, BL], F32)
            attn = consts.tile([1, BL * LT], F32)

            # Pre-broadcast overflow bias [p, ht*w] on the idle DVE during
            # the DMA wave, so the hot loop adds it in ONE tensor_tensor op
            # per ht (four per-row tensor_scalar ops each pay the DVE's
            # fixed PSUM-access cost and stall the PE).
            biasb_sb = []
            for k, rk in enumerate(rem):
                w = BL * rk
                zl = consts.tile([P, rk], F32, tag=f"zl{k}")
                nc.gpsimd.memset(zl[:], 0.0)
                bb = consts.tile([P, HT * w], F32, tag=f"biasb{k}")
                for ht in range(HT):
                    for r in range(BL):
                        nc.vector.tensor_scalar_add(
                            bb[:, ds(ht * w + r * rk, rk)], zl[:],
                            bias2_sb[:, ds(ht * BL + r, 1)])
                biasb_sb.append(bb)

            # ---- overflow streams first: every row's full score segment
            # is ready before that row's main-stream softmax tail runs.
            off = main
            for k, rk in enumerate(rem):
                w = BL * rk
                psr = psrp.tile([P, w], F32, tag=f"psr{k}")
                pend = []
                for ht in range(HT):
                    par = parp.tile([P, w], F32, tag="par")
                    for it2 in range(IT2):
                        if USE_FP8:
                            nc.tensor.matmul(
                                par[:], wpair(w1_sb, ht, it2),
                                encr_sb[k][:, ds(2 * it2, 2), :],
                                start=(it2 == 0), stop=(it2 == IT2 - 1),
                                perf_mode=DR)
                        else:
                            for i in range(2):
                                nc.tensor.matmul(
                                    par[:],
                                    w1_sb[:, ds(((ht * IT2 + it2) * 2 + i) * P, P)],
                                    encr_sb[k][:, ds(2 * it2 + i, 1), :],
                                    start=(it2 == 0 and i == 0),
                                    stop=(it2 == IT2 - 1 and i == 1))
                    # pre-broadcast bias in one DVE op; tanh on ACT
                    tp = thp.tile([P, w], F32, tag="tpre")
                    nc.vector.tensor_add(tp[:], par[:],
                                         biasb_sb[k][:, ds(ht * w, w)])
                    th = thp.tile([P, w], BF16, tag="thr")
                    nc.scalar.activation(th[:], tp[:], AF.Tanh,
                                         scale=ACT_SCALE)
                    pend.append((th, ht))
                    if len(pend) > 2:
                        pth, pht = pend.pop(0)
                        nc.tensor.matmul(psr[:], w2_sb[:, ds(pht * P, P)],
                                         pth[:], start=(pht == 0),
                                         stop=(pht == HT - 1))
                for pth, pht in pend:
                    nc.tensor.matmul(psr[:], w2_sb[:, ds(pht * P, P)], pth[:],
                                     start=(pht == 0), stop=(pht == HT - 1))
                for r in range(BL):
                    pos = r * LT + off
                    nc.vector.tensor_add(scores_sb[0:1, ds(pos, rk)],
                                         psr[0:1, ds(r * rk, rk)],
                                         pneg_sb[0:1, ds(pos, rk)])
                off += rk

            # ---- main streams: per batch row, with a single pending-
            # scores queue across rows so row-end drains interleave with
            # the next row's matmuls instead of bubbling the PE.
            def tail(r, psc):
                # softmax tail for row r (overflow scores already in).
                nc.vector.tensor_add(scores_sb[0:1, ds(r * LT, main)],
                                     psc[0:1, :],
                                     pneg_sb[0:1, ds(r * LT, main)])
                # |scores| <= ||w2||_1 <= 16, so exp(s - 40) never overflows
                # and softmax is shift-invariant -- no max-reduce needed.
                nc.scalar.activation(exps[0:1, ds(r * LT, LT)],
                                     scores_sb[0:1, ds(r * LT, LT)],
                                     AF.Exp, bias=c40[0:1, 0:1], scale=1.0,
                                     accum_out=ssum[0:1, ds(r, 1)])
                nc.vector.reciprocal(rcp[0:1, ds(r, 1)], ssum[0:1, ds(r, 1)])
                nc.vector.tensor_scalar_mul(attn[0:1, ds(r * LT, LT)],
                                            exps[0:1, ds(r * LT, LT)],
                                            rcp[0:1, ds(r, 1)])
                nc.sync.dma_start(out_ext[r, :], attn[0:1, ds(r * LT, LT)])

            pend = []

            def pop_scores():
                pth, pht, ppsc, prow = pend.pop(0)
                nc.tensor.matmul(ppsc[:], w2_sb[:, ds(pht * P, P)], pth[:],
                                 start=(pht == 0), stop=(pht == HT - 1))
                if pht == HT - 1:
                    tail(prow, ppsc)

            for r in range(BL):
                psc = pscp.tile([P, main], F32, tag="psc")
                for ht in range(HT):
                    pa = pap.tile([P, main], F32, tag="pa")
                    for it2 in range(IT2):
                        if USE_FP8:
                            nc.tensor.matmul(
                                pa[:], wpair(w1_sb, ht, it2),
                                encm_sb[r][:, ds(2 * it2, 2), :],
                                start=(it2 == 0), stop=(it2 == IT2 - 1),
                                perf_mode=DR)
                        else:
                            for i in range(2):
                                nc.tensor.matmul(
                                    pa[:],
                                    w1_sb[:, ds(((ht * IT2 + it2) * 2 + i) * P, P)],
                                    encm_sb[r][:, ds(2 * it2 + i, 1), :],
                                    start=(it2 == 0 and i == 0),
                                    stop=(it2 == IT2 - 1 and i == 1))
                    th = thp.tile([P, main], BF16, tag="th")
                    nc.scalar.activation(th[:], pa[:], AF.Tanh,
                                         bias=bias_sb[:, ds(ht * BL + r, 1)],
                                         scale=ACT_SCALE)
                    pend.append((th, ht, psc, r))
                    if len(pend) > DELAY:
                        pop_scores()
            while pend:
                pop_scores()

    nc.compile()
    _cached[key] = nc
    return nc


def _to_dev_dtype(a):
    if USE_FP8:
        return np.clip(a, -240.0, 240.0).astype(F8NP)
    return a.astype(BF)


def kernel(hidden, encoder_outputs, mask, W1, b1, W2, b2):
    global LAST_RESULT

    mask = np.asarray(mask, dtype=bool)
    idx = [np.nonzero(~mask[b])[0] for b in range(B)]
    cnt = np.array([len(i) for i in idx])
    main, rem = _geometry(int(cnt.max()))
    LT = main + sum(rem)
    nc = _build(main, rem)

    enc = np.asarray(encoder_outputs, dtype=np.float32)
    enc_t = np.transpose(enc, (1, 2, 0))            # [B, Hin, S]
    W1 = np.asarray(W1, dtype=np.float32)
    w1e = W1[:, :HIN].T                              # [Hin, H]
    w1h = W1[:, HIN:]                                # [H, H]
    hb = (np.asarray(hidden, np.float32) @ w1h.T
          + np.asarray(b1, np.float32).reshape(1, H))  # [B, H]
    w2 = np.asarray(W2, dtype=np.float32).reshape(H)

    # W1_enc.T packed for DoubleRow: [p, ht, it2, two, m]
    w1s = (w1e * W_SCALE) if USE_FP8 else w1e
    w1p = _to_dev_dtype(w1s).reshape(IT2, 2, P, HT, P)
    w1p = np.ascontiguousarray(np.transpose(w1p, (2, 3, 0, 1, 4))).reshape(P, -1)

    w2pad = np.zeros((P, HT, P), dtype=BF)
    w2pad[:, :, 0] = w2.reshape(HT, P).T
    w2pad = w2pad.reshape(P, HT * P)

    # packed enc per row + padneg
    encs = enc_t * ENC_SCALE if USE_FP8 else enc_t
    edt = F8NP if USE_FP8 else BF
    encm = np.zeros((B, HIN, main), dtype=edt)
    encr = [np.zeros((N_CORES, HIN, BL * rk), dtype=edt) for rk in rem]
    pneg = np.full((B, LT), np.float32(-1e30), dtype=np.float32)
    for b in range(B):
        c, rloc = divmod(b, BL)
        cols = _to_dev_dtype(encs[b][:, idx[b]])
        n = cnt[b]
        nm = min(n, main)
        encm[b, :, :nm] = cols[:, :nm]
        pneg[b, :n] = 0.0
        off = main
        for k, rk in enumerate(rem):
            if n > off:
                w = min(n - off, rk)
                encr[k][c, :, rloc * rk:rloc * rk + w] = cols[:, off:off + w]
            off += rk

    # [c, p, ht*BL] per-(h,b) bias
    biasT = np.ascontiguousarray(
        np.transpose(hb.reshape(N_CORES, BL, HT, P), (0, 3, 2, 1))
    ).reshape(N_CORES, P, HT * BL).astype(np.float32)
    bias2 = (biasT / ACT_SCALE).astype(np.float32)

    in_maps = []
    for c in range(N_CORES):
        sl = slice(c * BL, (c + 1) * BL)
        m = {
            "encm": np.ascontiguousarray(encm[sl]),
            "w1p": w1p,
            "biasT": biasT[c],
            "bias2": bias2[c],
            "w2pad": w2pad,
            "padneg": np.ascontiguousarray(pneg[sl].reshape(-1)),
        }
        for k in range(len(rem)):
            m[f"encr{k}"] = np.ascontiguousarray(encr[k][c])
        in_maps.append(m)

    res = run_bass_kernel_spmd(nc, in_maps, core_ids=list(range(N_CORES)))
    LAST_RESULT = res

    out = np.zeros((B, S), dtype=np.float32)
    for b in range(B):
        c, rloc = divmod(b, BL)
        row = res.results[c]["out"][rloc]
        out[b, idx[b]] = row[:cnt[b]]
    return np.ascontiguousarray(out[:, None, :])
